# revision 1
# baseline (speedup 1.0000x reference)
"""Trainium2 Bass kernel for nn_Encoder_8718783611479.

Reference computation (per the original nn.Module):
    xt = transpose(x, (0,3,1,2)).reshape(B, T, 180)          # B=2048, T=240
    gates = xt @ W_ih.T + b_ih + b_hh                        # (B, T, 360)
    i, f, g, o = split(gates, 4)                             # f unused (c0=0)
    c = sigmoid(i) * tanh(g)
    h = sigmoid(o) * tanh(c)
    out = sigmoid(h)                                         # (B, T, 90) f32

Design notes:
  * The LSTMCell is stateless per timestep -> one big GEMM over (B*T, 180).
  * x[b] viewed as (180, 240) is ALREADY the transposed stationary operand
    (lhsT = [K, M]) the tensor engine wants; the reference's transpose is
    absorbed into the matmul for free.
  * The f gate is dead; only 270 of 360 gate columns are computed.
  * sigmoid(z) = (1 + tanh(z/2)) / 2, so one unified ACT pass
    T = tanh(0.5 * G) over all 270 gate columns serves both the sigmoid
    (i, o) and tanh (g) gates -- the g columns' weights/bias are pre-scaled
    by 2 on the host so tanh(0.5 * 2g) = tanh(g).
  * Bias is folded into the matmul via an appended ones-row (K: 90 + 91).
  * Final sigmoid runs as a degree-3 odd polynomial on the vector engine
    (|h| <= 0.77, max err 7e-5) to offload the scalar engine.
  * Host repacks x into two planar [row, batch*col] bf16 arrays so every
    DMA descriptor is a >=512B contiguous run; the device emits output in
    partition-major [128, tile, 90] layout and the host un-permutes.

Sharding: pure data parallel, batch 2048 -> 8 cores x 256.
"""

import threading

import numpy as np
import ml_dtypes

import concourse.bass as bass
import concourse.mybir as mybir
import concourse.tile as tile
from concourse.vector_clock import ScopedClock
from concourse.bass_utils import run_bass_kernel_spmd

BF16 = ml_dtypes.bfloat16

# ---- problem constants (hardcoded per contract) ----
B, T, D_IN, H = 2048, 240, 180, 90
N_CORES = 8
BC = B // N_CORES          # 256 batches per core
NG = 270                   # live gate columns (i, g, o)

# ---- tiling config ----
NB = 16                    # batches per super-tile
ROWS = NB * T              # 3840 rows per super-tile
MT = ROWS // 128           # 30 m-tiles of 128 rows per super-tile
GRP = 3                    # m-tiles per PSUM group (3 banks/tile, 2 bufs = 6 banks)
NSUP = BC // NB            # super-tiles per core
NMT = BC * T // 128        # 480 m-tiles per core
SIGMA_POLY = True          # final sigmoid: False = ACT, True = DVE poly
OUT_BF16 = False           # device output dtype

_A0, _A1 = 0.12492107182371918, -0.0024438908399619275  # sigma poly coeffs


def _patched_lower_ordered_insts(self, ordered):
    # The installed walrus accepts at most ONE sync wait per ISA instruction.
    # Hoist overflow waits onto dedicated NOPs on the same engine, placed
    # immediately before the instruction (same-engine waits serialize, so
    # semantics are identical to a multi-wait instruction).
    for bb_name, insts in ordered.items():
        new = []
        for inst in insts:
            si = getattr(inst, "sync_info", None)
            if si is not None and len(si.on_wait) > 1:
                waits = list(si.on_wait)
                for w in waits[:-1]:
                    nop = mybir.InstNoOp(
                        name=self.nc.get_next_instruction_name(),
                        sync_info=mybir.SyncInfo(on_wait=[w], on_update=[]),
                        bass_nofuse=True,
                        engine=inst.engine,
                    )
                    new.append(nop)
                inst.sync_info = mybir.SyncInfo(
                    on_wait=[waits[-1]], on_update=list(si.on_update)
                )
            new.append(inst)
        insts[:] = new
    return _orig_lower_ordered_insts(self, ordered)


def _patched_drain_and_barrier(self, tick_clock, wait_clock):
    # Same single-wait constraint for the kernel-tail drain.
    drain_inst = self.nc.sync.drain()
    wait_clock.add_sem_waits(
        drain_inst.ins, ScopedClock({None: tick_clock.global_clock})
    )
    si = drain_inst.ins.sync_info
    if si is not None and len(si.on_wait) > 1:
        waits = list(si.on_wait)
        drain_inst.ins.sync_info = mybir.SyncInfo(
            on_wait=waits[:1], on_update=list(si.on_update)
        )
        for i in range(1, len(waits)):
            nop = self.nc.sync.nop(nofuse=True)
            nop.ins.sync_info = mybir.SyncInfo(on_wait=[waits[i]], on_update=[])
    self.nc.all_engine_barrier()
    assert self.sems is not None
    popped = self.nc._tile_sem_poison_stack.pop()
    assert popped is self._sem_poison
    self.nc.clear_and_free_semaphores(list(self.sems.allocated().values()))
    self.nc.all_engine_barrier()


if not getattr(tile.TileContext, "_single_wait_patched", False):
    tile.TileContext._orig_lower_unpatched = tile.TileContext._lower_ordered_insts
    tile.TileContext._lower_ordered_insts = _patched_lower_ordered_insts
    tile.TileContext._drain_and_barrier = _patched_drain_and_barrier
    tile.TileContext._single_wait_patched = True
_orig_lower_ordered_insts = tile.TileContext._orig_lower_unpatched


def build_nc(
    sigma_poly=None,
    out_bf16=None,
    grp4=False,
    bufs_x=3,
    bufs_t=2,
    bufs_ep=6,
    bufs_o=2,
    ep_chunks=1,
    super_plan=None,
    repeat=1,
    pool_offload=False,
    loop_repeat=None,
    tanh_dve_tiles=0,
    q_early=False,
    out_dma_act=False,
    bufs_ps=2,
    grp_sz=None,
    alloc_mode="stack",
    w_dma_act=False,
    tanhc_split=1,
):
    sigma_poly = SIGMA_POLY if sigma_poly is None else sigma_poly
    out_bf16 = OUT_BF16 if out_bf16 is None else out_bf16
    f32 = mybir.dt.float32
    bf16 = mybir.dt.bfloat16
    odt = {"f32": f32, "bf16": bf16, "f16": mybir.dt.float16}[
        out_bf16 if isinstance(out_bf16, str) else ("bf16" if out_bf16 else "f32")
    ]
    FT = mybir.ActivationFunctionType

    nc = bass.Bass()
    # xp1: x rows 0:90, planar [row, batch*col]; xp2: rows 90:180 + ones row
    xp1 = nc.declare_dram_parameter("xp1", [90, BC * T], bf16, isOutput=False)
    xp2 = nc.declare_dram_parameter("xp2", [91, BC * T], bf16, isOutput=False)
    wt = nc.declare_dram_parameter("wt", [D_IN + 1, NG], bf16, isOutput=False)
    # partition-major output: [partition, global m-tile, H]
    out = nc.declare_dram_parameter("out", [128, NMT, H], odt, isOutput=True)

    with tile.TileContext(nc, pool_alloc_mode=alloc_mode) as tc:
        with (
            tc.tile_pool(name="w", bufs=1) as wpool,
            tc.tile_pool(name="x", bufs=bufs_x) as xpool,
            tc.tile_pool(name="t", bufs=bufs_t) as tpool,
            tc.tile_pool(name="ep", bufs=bufs_ep) as eppool,
            tc.tile_pool(name="o", bufs=bufs_o) as opool,
            tc.tile_pool(name="ps", bufs=bufs_ps, space="PSUM") as pspool,
        ):
            wdma = nc.scalar if w_dma_act else nc.sync
            rhs1 = wpool.tile([90, NG], bf16)
            wdma.dma_start(rhs1[:], wt[0:90, :])
            rhs2 = wpool.tile([91, NG], bf16)
            wdma.dma_start(rhs2[:], wt[90:181, :])

            if super_plan is None:
                plan = [(NB, ep_chunks)] * NSUP
            else:
                plan = super_plan
            assert sum(nb for nb, _ in plan) == BC
            import contextlib
            loop_ctx = (
                tc.For_i(0, loop_repeat, 1)
                if loop_repeat is not None
                else contextlib.nullcontext()
            )
            with loop_ctx:
             for _rep in range(repeat):
              c0 = 0
              mt_base = 0
              for j, (nb_j, epc_j) in enumerate(plan):
                rows_j = nb_j * T
                mt_j = rows_j // 128
                xc1 = xpool.tile([90, ROWS], bf16, tag="xc1")
                nc.sync.dma_start(xc1[:, 0:rows_j], xp1[:, c0 : c0 + rows_j])
                xc2 = xpool.tile([91, ROWS], bf16, tag="xc2")
                nc.sync.dma_start(xc2[:, 0:rows_j], xp2[:, c0 : c0 + rows_j])

                # gate-class-major staging of tanh(0.5*G): [gate, mtile, col]
                Tt = tpool.tile([128, 3, MT, H], bf16, tag="Tt")
                if grp4:
                    groups, left = [], mt_j
                    while left >= 7 or left == 4:
                        groups.append(4)
                        left -= 4
                    while left:
                        groups.append(3)
                        left -= 3
                else:
                    gsz0 = grp_sz or GRP
                    assert mt_j % gsz0 == 0
                    groups = [gsz0] * (mt_j // gsz0)
                assert sum(groups) == mt_j
                m0 = 0
                for gsz in groups:
                    ps = pspool.tile([128, 4 if grp4 else (grp_sz or GRP), 512], f32, tag="ps")
                    for t in range(gsz):
                        m = m0 + t
                        nc.tensor.matmul(
                            ps[:, t, 0:NG],
                            xc1[:, 128 * m : 128 * (m + 1)],
                            rhs1[:],
                            start=True,
                            stop=False,
                        )
                        nc.tensor.matmul(
                            ps[:, t, 0:NG],
                            xc2[:, 128 * m : 128 * (m + 1)],
                            rhs2[:],
                            start=False,
                            stop=True,
                        )
                    nc.scalar.activation(
                        Tt[:, :, m0 : m0 + gsz, :].rearrange(
                            "p g t c -> p t g c"
                        ),
                        ps[:, 0:gsz, 0:NG].rearrange("p t (g c) -> p t g c", g=3),
                        FT.Tanh,
                        scale=0.5,
                    )
                    m0 += gsz

                for h2 in range(epc_j):
                    mlo = mt_j * h2 // epc_j
                    mhi = mt_j * (h2 + 1) // epc_j
                    F = (mhi - mlo) * H
                    Ts = Tt[:, :, mlo:mhi, :]
                    Ti = Ts[:, 0].rearrange("p t c -> p (t c)")
                    Tg = Ts[:, 1].rearrange("p t c -> p (t c)")
                    To = Ts[:, 2].rearrange("p t c -> p (t c)")

                    vadd = nc.gpsimd if pool_offload else nc.vector
                    P = eppool.tile([128, F], bf16, tag="ep")
                    vadd.tensor_scalar_add(P[:], Ti, 1.0)
                    if q_early:
                        Q = eppool.tile([128, F], bf16, tag="ep")
                        vadd.tensor_scalar_add(Q[:], To, 1.0)
                    C = eppool.tile([128, F], bf16, tag="ep")
                    nc.vector.tensor_mul(C[:], P[:], Tg)
                    U = eppool.tile([128, F], bf16, tag="ep")
                    # split tanh(0.5*C): ACT handles the first tiles, DVE a
                    # deg-5 odd minimax poly (|C|<=2, err<4e-4) on the rest,
                    # rebalancing the two engines
                    nmt_c = mhi - mlo
                    tdt = (
                        tanh_dve_tiles[j]
                        if isinstance(tanh_dve_tiles, (list, tuple))
                        else tanh_dve_tiles
                    )
                    kD = min((nmt_c * tdt) // MT, nmt_c) if tdt else 0
                    fA = (nmt_c - kD) * H
                    if fA > 0:
                        nsp = tanhc_split if fA % (tanhc_split * H) == 0 else 1
                        for sp in range(nsp):
                            lo, hi = fA * sp // nsp, fA * (sp + 1) // nsp
                            nc.scalar.activation(
                                U[:, lo:hi], C[:, lo:hi], FT.Tanh, scale=0.5
                            )
                    if kD > 0:
                        e0, e1, e2 = 0.49858181, -0.03849868, 0.00227525
                        Cs = C[:, fA:F]
                        FD = F - fA
                        v = eppool.tile([128, FD], bf16, tag="pv")
                        nc.vector.tensor_mul(v[:], Cs, Cs)
                        w = eppool.tile([128, FD], bf16, tag="pv")
                        nc.vector.tensor_scalar(
                            w[:], v[:], e2, e1,
                            mybir.AluOpType.mult, mybir.AluOpType.add,
                        )
                        w2 = eppool.tile([128, FD], bf16, tag="pv")
                        nc.vector.tensor_mul(w2[:], w[:], v[:])
                        w3 = eppool.tile([128, FD], bf16, tag="pv")
                        nc.vector.tensor_scalar_add(w3[:], w2[:], e0)
                        nc.vector.tensor_mul(U[:, fA:F], w3[:], Cs)
                    if not q_early:
                        Q = eppool.tile([128, F], bf16, tag="ep")
                        vadd.tensor_scalar_add(Q[:], To, 1.0)
                    Hh = eppool.tile([128, F], bf16, tag="ep")
                    nc.vector.tensor_mul(Hh[:], Q[:], U[:])

                    OUT = opool.tile([128, F], odt)
                    if sigma_poly:
                        # sigma(0.5*Hh) = 0.5 + Hh*(a0 + a1*Hh^2), |err| < 7e-5
                        W2 = eppool.tile([128, F], bf16, tag="ep")
                        (nc.gpsimd if pool_offload else nc.vector).tensor_mul(
                            W2[:], Hh[:], Hh[:]
                        )
                        V = eppool.tile([128, F], bf16, tag="ep")
                        nc.vector.tensor_scalar(
                            V[:], W2[:], _A1, _A0,
                            mybir.AluOpType.mult, mybir.AluOpType.add,
                        )
                        Z = eppool.tile([128, F], bf16, tag="ep")
                        nc.vector.tensor_mul(Z[:], V[:], Hh[:])
                        nc.vector.tensor_scalar_add(OUT[:], Z[:], 0.5)
                    else:
                        nc.scalar.activation(OUT[:], Hh[:], FT.Sigmoid, scale=0.5)

                    (nc.scalar if out_dma_act else nc.sync).dma_start(
                        out[:, mt_base + mlo : mt_base + mhi, :],
                        OUT[:].rearrange("p (t c) -> p t c", c=H),
                    )
                c0 += rows_j
                mt_base += mt_j
    return nc


_cache = threading.local()


DEFAULT_PLAN = [(8, 1)] * 2 + [(16, 1)] * 14 + [(8, 1)] * 2
DEFAULT_CFG = dict(out_bf16="f16", super_plan=DEFAULT_PLAN, bufs_t=3, tanh_dve_tiles=4, w_dma_act=True)


def _get_nc():
    nc = getattr(_cache, "nc", None)
    if nc is None:
        nc = build_nc(**DEFAULT_CFG)
        _cache.nc = nc
    return nc


def _prep_inputs(x, W_ih, W_hh, b_ih, b_hh):
    x = np.asarray(x, dtype=np.float32)
    W = np.asarray(W_ih, dtype=np.float32)
    b = np.asarray(b_ih, dtype=np.float32) + np.asarray(b_hh, dtype=np.float32)
    # gate order [i, g, o]; scale g by 2 for the unified tanh(0.5*G) pass
    W_eff = np.concatenate([W[0:90], 2.0 * W[180:270], W[270:360]], axis=0)
    b_eff = np.concatenate([b[0:90], 2.0 * b[180:270], b[270:360]], axis=0)
    wt = np.concatenate([W_eff.T, b_eff[None, :]], axis=0).astype(BF16)

    xr = x.reshape(B, D_IN, T).astype(BF16)       # (2048, 180, 240)
    # planar per-core packs: [row, batch*col] contiguous
    xp1s, xp2s = [], []
    for c in range(N_CORES):
        xc = xr[c * BC : (c + 1) * BC]            # (256, 180, 240)
        xp1s.append(
            np.ascontiguousarray(xc[:, 0:90, :].transpose(1, 0, 2)).reshape(
                90, BC * T
            )
        )
        x2 = np.empty((91, BC * T), dtype=BF16)
        x2[0:90] = np.ascontiguousarray(xc[:, 90:180, :].transpose(1, 0, 2)).reshape(
            90, BC * T
        )
        x2[90] = 1.0
        xp2s.append(x2)
    return xp1s, xp2s, wt


def kernel(x, W_ih, W_hh, b_ih, b_hh, _trace=False):
    xp1s, xp2s, wt = _prep_inputs(x, W_ih, W_hh, b_ih, b_hh)
    nc = _get_nc()
    in_maps = [
        {"xp1": xp1s[c], "xp2": xp2s[c], "wt": wt} for c in range(N_CORES)
    ]
    res = run_bass_kernel_spmd(nc, in_maps, list(range(N_CORES)), trace=_trace)
    # device out: [128, NMT, H], row (128*g + p) -> un-permute on host
    outs = []
    for c in range(N_CORES):
        o = res.results[c]["out"].astype(np.float32)
        outs.append(o.transpose(1, 0, 2).reshape(BC, T, H))
    if _trace:
        kernel._last_results = res
    return np.concatenate(outs, axis=0)

